# revision 1
# baseline (speedup 1.0000x reference)
"""Trainium2 Bass kernel for nn_DTAM (differential transposed-attention module).

Sharding: 8 cores = batch(4) x head(2). Each core computes its (b, h) shard
end-to-end; host does LayerNorm scale precompute, weight folding, and the
final partial-sum + residual merge (cheap O(B*C*N) work vs O(B*C^2*N) matmuls
on device).

Device pipeline per core (pixels processed in 8 super-chunks of 2048 px = 16
image rows):
  phase A: pw conv (PE) -> evac (ACT) -> depthwise 3x3: q,k on DVE (9-tap
           fused scalar_tensor_tensor chain), v on PE (diagonal-matmul with
           PSUM tap accumulation) -> DMA-xbar transpose of dwq/dwk (bf16) ->
           channel-attention score matmuls (PE, PSUM-accumulated over pixels)
  phase B: softmax halves, attn = attn1 - lam*attn2 (tiny [96,96] ops)
  phase C: y = attn @ dwv (PE), RMS stats (ones-matmul), r = exp(-0.5*ln(.))
           (ACT), r broadcast via K=1 matmul, out proj (PE), evac*r (DVE)
"""

import numpy as np
import ml_dtypes
from contextlib import ExitStack

BF16 = ml_dtypes.bfloat16

# ---- problem constants (hardcoded per contest rules) ----
B, C, H, W = 4, 192, 128, 128
HEADS = 2
N = H * W                 # 16384
HC = 96                   # half-channels per head (q1/q2 split)
LAM_INIT = 0.8
NSUP = 8                  # super-chunks
ROWS = 16                 # image rows per super-chunk
SUP = ROWS * W            # 2048 px
NCH = 4                   # 512-px chunks per super
CH = 512

_CACHED = {}


def _build_program():
    import concourse.bass as bass
    import concourse.bacc as bacc
    import concourse.tile as tile
    from concourse import mybir

    f32 = mybir.dt.float32
    bf16 = mybir.dt.bfloat16
    AF = mybir.ActivationFunctionType
    OP = mybir.AluOpType
    AX = mybir.AxisListType

    nc = bacc.Bacc("TRN2", target_bir_lowering=False, debug=False,
                   num_devices=8)

    # ---- DRAM I/O ----
    xs_a = nc.dram_tensor("xs_a", [128, N], bf16, kind="ExternalInput")
    xs_b = nc.dram_tensor("xs_b", [64, N], bf16, kind="ExternalInput")
    w_pw = {}
    for p in ("q", "k", "v"):
        w_pw[p] = (
            nc.dram_tensor(f"w{p}_a", [128, 192], bf16, kind="ExternalInput"),
            nc.dram_tensor(f"w{p}_b", [64, 192], bf16, kind="ExternalInput"),
        )
    wo_1 = nc.dram_tensor("wo_1", [96, 192], bf16, kind="ExternalInput")
    wo_2 = nc.dram_tensor("wo_2", [96, 192], bf16, kind="ExternalInput")
    tq_d = [nc.dram_tensor(f"tq{i}", [96, 9], f32, kind="ExternalInput")
            for i in (1, 2)]
    dk_d = [nc.dram_tensor(f"dk{i}", [96, 9, 96], bf16, kind="ExternalInput")
            for i in (1, 2)]
    dv_d = [nc.dram_tensor(f"dv{i}", [96, 9, 96], bf16, kind="ExternalInput")
            for i in (1, 2)]
    ones96_d = nc.dram_tensor("ones96", [96, 1], bf16, kind="ExternalInput")
    ones1_d = nc.dram_tensor("ones1", [1, 128], f32, kind="ExternalInput")
    ident_d = nc.dram_tensor("ident", [96, 96], bf16, kind="ExternalInput")
    neglam_d = nc.dram_tensor("neglam", [128, 1], f32, kind="ExternalInput")
    out_d = nc.dram_tensor("out", [192, N], f32, kind="ExternalOutput")

    # tap t in 0..8 -> spatial offset (oy, ox), correlation convention
    OFFS = [(t // 3 - 1, t % 3 - 1) for t in range(9)]
    # order taps so the full-range center tap comes first
    TAP_ORDER = [4] + [t for t in range(9) if t != 4]

    def xr(ox):
        # (out_slice, in_slice) ranges along x for offset ox
        if ox == -1:
            return (1, 128), (0, 127)
        if ox == 1:
            return (0, 127), (1, 128)
        return (0, 128), (0, 128)

    with tile.TileContext(nc) as tc, ExitStack() as ctx:
        cst = ctx.enter_context(tc.tile_pool(name="cst", bufs=1))
        res = ctx.enter_context(tc.tile_pool(name="res", bufs=1))

        # ---- load constants ----
        wt = {}
        for p in ("q", "k", "v"):
            ta = cst.tile([128, 192], bf16, name=f"w{p}a", tag=f"w{p}a")
            tb = cst.tile([64, 192], bf16, name=f"w{p}b", tag=f"w{p}b")
            nc.sync.dma_start(ta[:], w_pw[p][0][:])
            nc.sync.dma_start(tb[:], w_pw[p][1][:])
            wt[p] = (ta, tb)
        wo1 = cst.tile([96, 192], bf16, name="wo1", tag="wo1")
        wo2 = cst.tile([96, 192], bf16, name="wo2", tag="wo2")
        nc.sync.dma_start(wo1[:], wo_1[:])
        nc.sync.dma_start(wo2[:], wo_2[:])
        tq = [cst.tile([96, 9], f32, name=f"tq{i}", tag=f"tq{i}") for i in range(2)]
        dk = [cst.tile([96, 9, 96], bf16, name=f"dk{i}", tag=f"dk{i}") for i in range(2)]
        dv = [cst.tile([96, 9, 96], bf16, name=f"dv{i}", tag=f"dv{i}") for i in range(2)]
        for i in range(2):
            nc.sync.dma_start(tq[i][:], tq_d[i][:])
            nc.sync.dma_start(dk[i][:], dk_d[i][:])
            nc.sync.dma_start(dv[i][:], dv_d[i][:])
        ones96 = cst.tile([96, 1], bf16, name="o96", tag="o96")
        ones1 = cst.tile([1, 128], f32, name="o1", tag="o1")
        ident = cst.tile([96, 96], bf16, name="id", tag="id")
        neglam = cst.tile([128, 1], f32, name="nl", tag="nl")
        nc.sync.dma_start(ones96[:], ones96_d[:])
        nc.sync.dma_start(ones1[:], ones1_d[:])
        nc.sync.dma_start(ident[:], ident_d[:])
        nc.sync.dma_start(neglam[:], neglam_d[:])

        eps6 = cst.tile([1, 1], f32, name="eps6", tag="eps6")
        nc.vector.memset(eps6[:], 1e-6)

        # resident dwv halves
        dwv_res = [res.tile([96, N], bf16, name=f"dwv{i}", tag=f"dwv{i}") for i in range(2)]

        smx = ctx.enter_context(tc.tile_pool(name="smx", bufs=1))
        if True:
            # SBUF score accumulators (summed over supers)
            sc = [res.tile([96, 96], f32, name=f"sc{i}", tag=f"sc{i}") for i in range(2)]
            nc.vector.memset(sc[0][:], 0.0)
            nc.vector.memset(sc[1][:], 0.0)

            # ================= PHASE A =================
            with tc.tile_pool(name="xsp", bufs=2) as xsp, \
                 tc.tile_pool(name="qkvp", bufs=2) as qkvp, \
                 tc.tile_pool(name="dwo", bufs=2) as dwo, \
                 tc.tile_pool(name="tro", bufs=2) as tro, \
                 tc.tile_pool(name="pwps", bufs=3, space="PSUM") as pwps, \
                 tc.tile_pool(name="scps", bufs=1, space="PSUM") as scps_p, \
                 tc.tile_pool(name="dvps", bufs=2, space="PSUM") as dvps:

                sup_t = {}   # (s) -> dict of 6 halo'd super tiles
                prev = None

                for s in range(NSUP + 1):
                    if s < NSUP:
                        # ---- allocate halo'd super tiles for s ----
                        cur = {}
                        for p in ("q", "k", "v"):
                            for hf in range(2):
                                cur[(p, hf)] = qkvp.tile(
                                    [96, ROWS + 2, 128], bf16, name=f"{p}{hf}", tag=f"{p}{hf}")
                        if s == 0:
                            for p in ("q", "k", "v"):
                                nc.vector.memset(cur[(p, 0)][:, 0, :], 0.0)
                                nc.vector.memset(cur[(p, 1)][:, 0, :], 0.0)
                        if s == NSUP - 1:
                            for p in ("q", "k", "v"):
                                nc.vector.memset(cur[(p, 0)][:, ROWS + 1, :], 0.0)
                                nc.vector.memset(cur[(p, 1)][:, ROWS + 1, :], 0.0)
                        sup_t[s] = cur

                        # ---- load xs super ----
                        off = s * SUP
                        xa = xsp.tile([128, SUP], bf16, name="xa", tag="xa")
                        xb = xsp.tile([64, SUP], bf16, name="xb", tag="xb")
                        nc.sync.dma_start(xa[:], xs_a[:, off:off + SUP])
                        nc.sync.dma_start(xb[:], xs_b[:, off:off + SUP])

                        # ---- pointwise conv + evac ----
                        for cc in range(NCH):
                            c0 = cc * CH
                            for p in ("q", "k", "v"):
                                for mt in range(2):
                                    ps = pwps.tile([96, CH], f32, name="pw", tag="pw")
                                    nc.tensor.matmul(
                                        ps[:], wt[p][0][:, mt * 96:(mt + 1) * 96],
                                        xa[:, c0:c0 + CH],
                                        start=True, stop=False)
                                    nc.tensor.matmul(
                                        ps[:], wt[p][1][:, mt * 96:(mt + 1) * 96],
                                        xb[:, c0:c0 + CH],
                                        start=False, stop=True)
                                    # evac into interior rows of super tile
                                    dst = cur[(p, mt)][:, 1 + cc * 4:1 + cc * 4 + 4, :]
                                    src_ap = ps[:].rearrange(
                                        "p (r x) -> p r x", x=128)
                                    if p == "v":
                                        nc.vector.tensor_copy(dst, src_ap)
                                    else:
                                        nc.scalar.copy(dst, src_ap)

                        # ---- halo row copies between s-1 and s ----
                        if prev is not None:
                            for p in ("q", "k", "v"):
                                for hf in range(2):
                                    nc.gpsimd.tensor_copy(prev[(p, hf)][:, ROWS + 1, :],
                                                     cur[(p, hf)][:, 1, :])
                                    nc.gpsimd.tensor_copy(cur[(p, hf)][:, 0, :],
                                                     prev[(p, hf)][:, ROWS, :])

                    # ---- process super s-1 (halos complete) ----
                    if prev is not None:
                        sp = s - 1
                        off = sp * SUP
                        # depthwise q on DVE (9-tap STT chain)
                        dwqk = {}
                        for p, tp in (("q", tq),):
                            for hf in range(2):
                                src = prev[(p, hf)]
                                dst = dwo.tile([96, ROWS, 128], bf16,
                                               name=f"dw{p}{hf}", tag=f"dw{p}{hf}")
                                dwqk[(p, hf)] = dst
                                for ti, t in enumerate(TAP_ORDER):
                                    oy, ox = OFFS[t]
                                    (a0, a1), (b0, b1) = xr(ox)
                                    o_ap = dst[:, 0:ROWS, a0:a1]
                                    i_ap = src[:, 1 + oy:1 + oy + ROWS, b0:b1]
                                    scl = tp[hf][:, t:t + 1]
                                    if ti == 0:
                                        nc.vector.tensor_scalar(
                                            dst[:], src[:, 1:1 + ROWS, :],
                                            scl, None, OP.mult)
                                    else:
                                        nc.vector.scalar_tensor_tensor(
                                            o_ap, i_ap, scl, o_ap,
                                            OP.mult, OP.add)
                        # depthwise k, v on PE (diag matmuls, PSUM tap accum)
                        for hf in range(2):
                            dst_k = dwo.tile([96, ROWS, 128], bf16,
                                             name=f"dwk{hf}", tag=f"dwk{hf}")
                            dwqk[("k", hf)] = dst_k
                            src = prev[("k", hf)]
                            for cc in range(NCH):
                                rr = cc * 4
                                pk = dvps.tile([96, 4, 128], f32, name="dkp",
                                               tag="dkp")
                                for ti, t in enumerate(TAP_ORDER):
                                    oy, ox = OFFS[t]
                                    (a0, a1), (b0, b1) = xr(ox)
                                    nc.tensor.matmul(
                                        pk[:, :, a0:a1],
                                        dk[hf][:, t, :],
                                        src[:, 1 + rr + oy:1 + rr + oy + 4, b0:b1],
                                        start=(ti == 0), stop=(ti == 8))
                                nc.scalar.copy(dst_k[:, rr:rr + 4, :], pk[:])
                        for hf in range(2):
                            src = prev[("v", hf)]
                            for cc in range(NCH):
                                rr = cc * 4
                                pv = dvps.tile([96, 4, 128], f32, name="dv", tag="dv")
                                for ti, t in enumerate(TAP_ORDER):
                                    oy, ox = OFFS[t]
                                    (a0, a1), (b0, b1) = xr(ox)
                                    nc.tensor.matmul(
                                        pv[:, :, a0:a1],
                                        dv[hf][:, t, :],
                                        src[:, 1 + rr + oy:1 + rr + oy + 4, b0:b1],
                                        start=(ti == 0), stop=(ti == 8))
                                seg = off + cc * CH
                                nc.scalar.copy(
                                    dwv_res[hf][:, seg:seg + CH],
                                    pv[:].rearrange("p r x -> p (r x)"))
                        # DMA-xbar transposes of dwq/dwk -> [128, 16, 96]
                        trt = {}
                        for p in ("q", "k"):
                            for hf in range(2):
                                tt = tro.tile([128, ROWS, 96], bf16,
                                              name=f"t{p}{hf}", tag=f"t{p}{hf}")
                                trt[(p, hf)] = tt
                                nc.scalar.dma_start_transpose(
                                    tt[:], dwqk[(p, hf)][:].rearrange(
                                        "p r x -> p (r x)"))
                        # score matmuls (accumulate in PSUM per super, then
                        # fold into the SBUF accumulator)
                        for hf in range(2):
                            psc = scps_p.tile([96, 96], f32, name=f"psc{hf}",
                                              tag="psc")
                            for blk in range(ROWS):
                                nc.tensor.matmul(
                                    psc[:],
                                    trt[("q", hf)][:, blk, :],
                                    trt[("k", hf)][:, blk, :],
                                    start=(blk == 0),
                                    stop=(blk == ROWS - 1))
                            nc.vector.tensor_tensor(sc[hf][:], sc[hf][:],
                                                    psc[:], OP.add)
                    if s < NSUP:
                        prev = sup_t[s]

            # ================= PHASE B: softmax + attn =================
            atstack = ExitStack()
            atps = atstack.enter_context(
                tc.tile_pool(name="atps", bufs=1, space="PSUM"))
            if True:
                ex = []
                rr_ = []
                for hf in range(2):
                    nm = smx.tile([96, 1], f32, name=f"nm{hf}", tag=f"nm{hf}")
                    nc.vector.tensor_reduce(nm[:], sc[hf][:], AX.X, OP.max,
                                            negate=True)
                    e = smx.tile([96, 96], f32, name=f"e{hf}", tag=f"e{hf}")
                    nc.scalar.activation(e[:], sc[hf][:], AF.Exp, bias=nm[:, 0:1])
                    sm = smx.tile([96, 1], f32, name=f"sm{hf}", tag=f"sm{hf}")
                    nc.vector.tensor_reduce(sm[:], e[:], AX.X, OP.add)
                    r = smx.tile([96, 1], f32, name=f"r{hf}", tag=f"r{hf}")
                    nc.vector.reciprocal(r[:], sm[:])
                    ex.append(e)
                    rr_.append(r)
                r2n = smx.tile([96, 1], f32, name="r2n", tag="r2n")
                nc.vector.tensor_scalar(r2n[:], rr_[1][:], neglam[0:96, 0:1],
                                        None, OP.mult)
                a1 = smx.tile([96, 96], f32, name="a1", tag="a1")
                nc.scalar.mul(a1[:], ex[0][:], rr_[0][:, 0:1])
                attn = smx.tile([96, 96], bf16, name="attn", tag="attn")
                nc.vector.scalar_tensor_tensor(attn[:], ex[1][:], r2n[:, 0:1],
                                               a1[:], OP.mult, OP.add)
                pt = atps.tile([96, 96], bf16, name="pt", tag="pt")
                nc.tensor.transpose(pt[:], attn[:], ident[:])
                attnT = smx.tile([96, 96], bf16, name="attnT", tag="attnT")
                nc.scalar.copy(attnT[:], pt[:])
                atstack.close()

                # ================= PHASE C =================
                with tc.tile_pool(name="yp", bufs=2) as yp, \
                     tc.tile_pool(name="op_", bufs=2) as op_, \
                     tc.tile_pool(name="yps", bufs=2, space="PSUM") as yps, \
                     tc.tile_pool(name="sqps", bufs=1, space="PSUM") as sqps, \
                     tc.tile_pool(name="rbps", bufs=1, space="PSUM") as rbps, \
                     tc.tile_pool(name="ops", bufs=2, space="PSUM") as ops:
                    for cc in range(N // CH):
                        seg = cc * CH
                        ysb = []
                        yyb = []
                        for hf in range(2):
                            py = yps.tile([96, CH], f32, name=f"y{hf}", tag=f"y{hf}")
                            nc.tensor.matmul(py[:], attnT[:],
                                             dwv_res[hf][:, seg:seg + CH],
                                             start=True, stop=True)
                            ys = yp.tile([96, CH], bf16, name=f"ys{hf}", tag=f"ys{hf}")
                            nc.scalar.copy(ys[:], py[:])
                            yy = yp.tile([96, CH], bf16, name=f"yy{hf}", tag=f"yy{hf}")
                            nc.gpsimd.tensor_tensor(yy[:], ys[:], ys[:], OP.mult)
                            ysb.append(ys)
                            yyb.append(yy)
                        pss = sqps.tile([1, CH], f32, name="ss", tag="ss")
                        nc.tensor.matmul(pss[:], ones96[:], yyb[0][:],
                                         start=True, stop=False)
                        nc.tensor.matmul(pss[:], ones96[:], yyb[1][:],
                                         start=False, stop=True)
                        rsb = op_.tile([1, CH], f32, name="rs", tag="rs")
                        nc.scalar.activation(rsb[:], pss[:],
                                             AF.Abs_reciprocal_sqrt,
                                             bias=eps6[0:1, 0:1],
                                             scale=1.0 / 192.0)
                        prb = rbps.tile([128, CH], f32, name="rb", tag="rb")
                        nc.tensor.matmul(prb[:], ones1[:], rsb[:],
                                         start=True, stop=True)
                        rbsb = op_.tile([128, CH], f32, name="rbs", tag="rbs")
                        nc.vector.tensor_copy(rbsb[:], prb[:])
                        for mt in range(2):
                            po = ops.tile([96, CH], f32, name="po", tag="po")
                            nc.tensor.matmul(po[:], wo1[:, mt * 96:(mt + 1) * 96],
                                             ysb[0][:], start=True, stop=False)
                            nc.tensor.matmul(po[:], wo2[:, mt * 96:(mt + 1) * 96],
                                             ysb[1][:], start=False, stop=True)
                            osb = op_.tile([96, CH], f32, name=f"os{mt}", tag=f"os{mt}")
                            nc.vector.tensor_tensor(osb[:], po[:],
                                                    rbsb[0:96, :], OP.mult)
                            nc.sync.dma_start(
                                out_d[mt * 96:(mt + 1) * 96, seg:seg + CH],
                                osb[:])
    nc.compile()
    return nc


def _prep_inputs(inputs):
    x = np.asarray(inputs["x"], np.float32)
    norm_w = np.asarray(inputs["norm_w"], np.float32)
    Wq = np.asarray(inputs["Wq"], np.float32)
    Wk = np.asarray(inputs["Wk"], np.float32)
    Wv = np.asarray(inputs["Wv"], np.float32)
    Dq = np.asarray(inputs["Dq"], np.float32)
    Dk = np.asarray(inputs["Dk"], np.float32)
    Dv = np.asarray(inputs["Dv"], np.float32)
    t1 = np.asarray(inputs["t1"], np.float32)
    t2 = np.asarray(inputs["t2"], np.float32)
    hn_w = np.asarray(inputs["hn_w"], np.float32)
    Wo = np.asarray(inputs["Wo"], np.float32)
    lam = float(np.exp(np.sum(inputs["lq1"] * inputs["lk1"], dtype=np.float64))
                - np.exp(np.sum(inputs["lq2"] * inputs["lk2"], dtype=np.float64))
                + LAM_INIT)

    # LayerNorm scale on host
    var = x.var(axis=1)                       # [B, H, W]
    s = 1.0 / np.sqrt(var + 1e-5)
    xs = (x * s[:, None, :, :]).reshape(B, C, N)

    Wq_f = Wq * norm_w[None, :]
    Wk_f = Wk * norm_w[None, :]
    Wv_f = Wv * norm_w[None, :]

    in_maps = []
    for core in range(8):
        b, h = core // 2, core % 2
        sl = slice(h * 192, (h + 1) * 192)
        m = {}
        m["xs_a"] = xs[b, 0:128].astype(BF16)
        m["xs_b"] = xs[b, 128:192].astype(BF16)
        for nm, Wf in (("q", Wq_f), ("k", Wk_f), ("v", Wv_f)):
            lhsT = Wf[sl].T.astype(BF16)      # [192 in, 192 out]
            m[f"w{nm}_a"] = np.ascontiguousarray(lhsT[0:128])
            m[f"w{nm}_b"] = np.ascontiguousarray(lhsT[128:192])
        dq = Dq[sl, 0].reshape(192, 9)
        dk = Dk[sl, 0].reshape(192, 9)
        dvv = Dv[sl, 0].reshape(192, 9)
        m["tq1"] = np.ascontiguousarray(dq[0:96] * t1[h, 0, 0])
        m["tq2"] = np.ascontiguousarray(dq[96:192] * t2[h, 0, 0])
        idx = np.arange(96)
        for i in (1, 2):
            dmat = np.zeros((96, 9, 96), np.float32)
            dmat[idx, :, idx] = dk[(i - 1) * 96:i * 96]
            m[f"dk{i}"] = dmat.astype(BF16)
            dmat = np.zeros((96, 9, 96), np.float32)
            dmat[idx, :, idx] = dvv[(i - 1) * 96:i * 96]
            m[f"dv{i}"] = dmat.astype(BF16)
        Wo_hf = Wo[:, sl] * (hn_w[h] * (1.0 - LAM_INIT))[None, :]
        lhsT = Wo_hf.T.astype(BF16)           # [192 y-ch, 192 out]
        m["wo_1"] = np.ascontiguousarray(lhsT[0:96])
        m["wo_2"] = np.ascontiguousarray(lhsT[96:192])
        m["ones96"] = np.ones((96, 1), BF16)
        m["ones1"] = np.ones((1, 128), np.float32)
        m["ident"] = np.eye(96, dtype=BF16)
        m["neglam"] = np.full((128, 1), -lam, np.float32)
        in_maps.append(m)
    return in_maps


def kernel(**inputs):
    from concourse import bass_utils

    if "nc" not in _CACHED:
        _CACHED["nc"] = _build_program()
    nc = _CACHED["nc"]

    in_maps = _prep_inputs(inputs)
    results = bass_utils.run_bass_kernel_spmd(
        nc, in_maps, core_ids=list(range(8))).results

    x = np.asarray(inputs["x"], np.float32)
    out = np.empty((B, C, N), np.float32)
    for b in range(B):
        out[b] = results[2 * b]["out"] + results[2 * b + 1]["out"]
    out = out.reshape(B, C, H, W) + x
    return out.astype(np.float32)



# revision 13
# speedup vs baseline: 1.3470x; 1.3470x over previous
"""Trainium2 Bass kernel for nn_DTAM (differential transposed-attention module).

Sharding: 8 cores = batch(4) x head(2); host merges head partial sums + residual.

Redesign vs baseline: the pointwise conv and 3x3 depthwise conv are FUSED into
9 per-tap weight matrices (host-folded), so dwq/dwk are computed directly from
a zero-padded fp8 copy of the LN-scaled input with 9 fp8 DoubleRow matmuls per
output half (contraction 192 = 2 k-tiles of 96).  The v path is fused all the
way through the attention matrix: y = sum_u ((attn * tv_u) @ Wv) @ xs_shift_u,
so V/dwv are never materialized.  All fp8 stages carry power-of-2 scales,
compensated in the softmax exp scale, the RMS epsilon scale, and the broadcast
ones-vector.

Pipeline per core:
  A: per 512-px chunk: 36 fp8-DR matmuls -> dwq/dwk psum; evac bf16 (ACT/DVE);
     per 2048-px group: DMA-xbar transpose; score matmuls accumulate in PSUM.
  B: softmax halves, attn = a1 - lam*a2, PE transpose, build 18 C_u = (attnT *
     tv_u)^T @ Wv matrices, evac to fp8.
  C: per chunk: 18 fp8-DR matmuls -> y psum; ysb fp8 evac; yy = y^2 (DVE);
     RMS stats (ones-matmul); r = rsqrt (ACT); r broadcast via K=1 matmul;
     out proj fp8-DR; osb = po * r (DVE/GPSIMD); DMA out bf16.
"""

import numpy as np
import ml_dtypes
from contextlib import ExitStack

BF16 = ml_dtypes.bfloat16
F8 = ml_dtypes.float8_e4m3

# ---- problem constants (hardcoded per contest rules) ----
B, C, H, W = 4, 192, 128, 128
HEADS = 2
N = H * W
LAM_INIT = 0.8
NCHUNK = 32          # 512-px chunks (4 image rows)
RPC = 4              # rows per chunk
GRP = 4              # chunks per transpose group
PW = W + 2           # padded width 130

# power-of-2 scales
SXS = 16.0           # xs fp8 scale
SW = 2.0 ** 16       # fused dw weight scale
SCU = 2.0 ** 18      # C_u fp8 scale
SWO = 2.0 ** 11      # Wo fp8 scale
SY = 2.0 ** -10      # ysb evac scale
SY0 = SXS * SCU      # y_psum scale
EXP_SCALE = 1.0 / (SXS * SW) ** 2
RSQ_SCALE = 1.0 / (192.0 * (SY0 * SY) ** 2)
FVAL = 1.0 / (SY0 * SY * SWO)   # folded into broadcast ones

# tap t -> (dy, dx)
OFFS = [(t // 3 - 1, t % 3 - 1) for t in range(9)]

_CACHED = {}


def _build_program():
    import concourse.bass as bass
    import concourse.bacc as bacc
    import concourse.tile as tile
    from concourse import mybir

    f32 = mybir.dt.float32
    bf16 = mybir.dt.bfloat16
    f8 = mybir.dt.float8e4
    AF = mybir.ActivationFunctionType
    OP = mybir.AluOpType
    AX = mybir.AxisListType
    DR = mybir.MatmulPerfMode.DoubleRow

    nc = bacc.Bacc("TRN2", target_bir_lowering=False, debug=False,
                   num_devices=8)

    # ---- DRAM I/O ----
    xs_d = nc.dram_tensor("xs8", [96, 2, H, W], f8, kind="ExternalInput")
    wq_d = nc.dram_tensor("wq8", [96, 9, 2, 2, 96], f8, kind="ExternalInput")
    wk_d = nc.dram_tensor("wk8", [96, 9, 2, 2, 96], f8, kind="ExternalInput")
    wv_d = nc.dram_tensor("wv_dm", [96, 2, 2, 96], bf16, kind="ExternalInput")
    dv_d = nc.dram_tensor("dv_t", [96, 2, 9], f32, kind="ExternalInput")
    wo_d = nc.dram_tensor("wo8", [96, 2, 2, 96], f8, kind="ExternalInput")
    id_d = nc.dram_tensor("ident", [96, 96], bf16, kind="ExternalInput")
    o96_d = nc.dram_tensor("ones96", [96, 1], bf16, kind="ExternalInput")
    oF_d = nc.dram_tensor("onesF", [1, 96], bf16, kind="ExternalInput")
    nl_d = nc.dram_tensor("neglam", [96, 1], f32, kind="ExternalInput")
    eps_d = nc.dram_tensor("eps", [1, 1], f32, kind="ExternalInput")
    out_d = nc.dram_tensor("out", [2, 96, N], bf16, kind="ExternalOutput")

    with tile.TileContext(nc) as tc, ExitStack() as ctx:
        cst = ctx.enter_context(tc.tile_pool(name="cst", bufs=1))
        res = ctx.enter_context(tc.tile_pool(name="res", bufs=1))

        # ---- constants ----
        wq8 = cst.tile([96, 9, 2, 2, 96], f8, name="wq8", tag="wq8")
        wk8 = cst.tile([96, 9, 2, 2, 96], f8, name="wk8", tag="wk8")
        wv_dm = cst.tile([96, 2, 2, 96], bf16, name="wvdm", tag="wvdm")
        dv_t = cst.tile([96, 2, 9], f32, name="dvt", tag="dvt")
        wo8 = cst.tile([96, 2, 2, 96], f8, name="wo8", tag="wo8")
        ident = cst.tile([96, 96], bf16, name="id", tag="id")
        ones96 = cst.tile([96, 1], bf16, name="o96", tag="o96")
        onesF = cst.tile([1, 96], bf16, name="oF", tag="oF")
        neglam = cst.tile([96, 1], f32, name="nl", tag="nl")
        eps = cst.tile([1, 1], f32, name="eps", tag="eps")
        for t_, d_ in ((wq8, wq_d), (wk8, wk_d), (wv_dm, wv_d), (dv_t, dv_d),
                       (wo8, wo_d), (ident, id_d), (ones96, o96_d),
                       (onesF, oF_d), (neglam, nl_d), (eps, eps_d)):
            nc.sync.dma_start(t_[:], d_[:])

        # ---- padded fp8 input [96, 2, 130, 130] ----
        xs = res.tile([96, 2, H + 2, PW], f8, name="xs", tag="xs")
        nc.vector.memset(xs[:, :, 0, :], 0.0)
        nc.vector.memset(xs[:, :, H + 1, :], 0.0)
        nc.vector.memset(xs[:, :, 1:H + 1, 0:1], 0.0)
        nc.vector.memset(xs[:, :, 1:H + 1, W + 1:W + 2], 0.0)
        for i in range(8):
            r0 = 16 * i
            for j in range(2):
                nc.sync.dma_start(xs[:, j, 1 + r0:1 + r0 + 16, 1:W + 1],
                                  xs_d[:, j, r0:r0 + 16, :])

        # score accumulator lives in PSUM across phases A+B
        scp_ctx = ExitStack()
        scp = scp_ctx.enter_context(tc.tile_pool(name="scp", bufs=1, space="PSUM"))
        psc = scp.tile([96, 2, 512], f32, name="psc", tag="psc")

        # ================= PHASE A =================
        with tc.tile_pool(name="dwps", bufs=3, space="PSUM") as dwps, \
             tc.tile_pool(name="dws", bufs=2) as dws, \
             tc.tile_pool(name="trp", bufs=2) as trp:
            for g in range(8):
                q_sb = dws.tile([96, 2, 16, 128], bf16, name="qsb", tag="qsb")
                k_sb = dws.tile([96, 2, 16, 128], bf16, name="ksb", tag="ksb")
                for cc in range(GRP):
                    c = g * GRP + cc
                    r = RPC * c
                    tq_ps = dwps.tile([96, 2, RPC, 128], f32, name="tq",
                                      tag="dwps")
                    tk_ps = dwps.tile([96, 2, RPC, 128], f32, name="tk",
                                      tag="dwps")
                    for w8, ps in ((wq8, tq_ps), (wk8, tk_ps)):
                        for hf in range(2):
                            for t in range(9):
                                dy, dx = OFFS[t]
                                nc.tensor.matmul(
                                    ps[:, hf],
                                    w8[:, t, :, hf, :],
                                    xs[:, :, 1 + r + dy:1 + r + dy + RPC,
                                       1 + dx:1 + dx + W],
                                    start=(t == 0), stop=(t == 8),
                                    perf_mode=DR)
                    for hf in range(2):
                        nc.scalar.copy(q_sb[:, hf, RPC * cc:RPC * cc + RPC, :],
                                       tq_ps[:, hf])
                        nc.vector.tensor_copy(
                            k_sb[:, hf, RPC * cc:RPC * cc + RPC, :],
                            tk_ps[:, hf])
                # DMA-xbar transposes -> [128, 16, 96]
                trts = {}
                for nm, sb in (("q", q_sb), ("k", k_sb)):
                    for hf in range(2):
                        tt = trp.tile([128, 16, 96], bf16, name=f"t{nm}{hf}",
                                      tag=f"t{nm}{hf}")
                        trts[(nm, hf)] = tt
                        nc.sync.dma_start_transpose(tt[:], sb[:, hf])
                # score matmuls (PSUM-accumulated across all groups)
                for hf in range(2):
                    for blk in range(16):
                        nc.tensor.matmul(
                            psc[:, hf, 0:96],
                            trts[("q", hf)][:, blk, :],
                            trts[("k", hf)][:, blk, :],
                            start=(g == 0 and blk == 0),
                            stop=(g == 7 and blk == 15))

        # ================= PHASE B =================
        smx = ctx.enter_context(tc.tile_pool(name="smx", bufs=1))
        C8 = res.tile([96, 2, 2, 9, 96], f8, name="C8", tag="C8")
        with tc.tile_pool(name="bps", bufs=2, space="PSUM") as bps:
            nm_t = smx.tile([96, 2, 1], f32, name="nm", tag="nm")
            nms = smx.tile([96, 2, 1], f32, name="nms", tag="nms")
            e_t = smx.tile([96, 2, 96], f32, name="e", tag="e")
            sm_t = smx.tile([96, 2, 1], f32, name="sm", tag="sm")
            rr_t = smx.tile([96, 2, 1], f32, name="rr", tag="rr")
            for hf in range(2):
                nc.vector.tensor_reduce(nm_t[:, hf], psc[:, hf, 0:96], AX.X, OP.max,
                                        negate=True)
            nc.vector.tensor_scalar(nms[:], nm_t[:], EXP_SCALE, None, OP.mult)
            for hf in range(2):
                nc.scalar.activation(e_t[:, hf], psc[:, hf, 0:96], AF.Exp,
                                     bias=nms[:, hf], scale=EXP_SCALE)
                nc.vector.tensor_reduce(sm_t[:, hf], e_t[:, hf], AX.X, OP.add)
            nc.vector.reciprocal(rr_t[:], sm_t[:])
            r2n = smx.tile([96, 1], f32, name="r2n", tag="r2n")
            nc.vector.tensor_scalar(r2n[:], rr_t[:, 1], neglam[:, 0:1], None,
                                    OP.mult)
            a1 = smx.tile([96, 96], f32, name="a1", tag="a1")
            nc.scalar.mul(a1[:], e_t[:, 0], rr_t[:, 0, 0:1])
            attn = smx.tile([96, 96], bf16, name="attn", tag="attn")
            nc.vector.scalar_tensor_tensor(attn[:], e_t[:, 1], r2n[:, 0:1],
                                           a1[:], OP.mult, OP.add)
            pt = bps.tile([96, 96], bf16, name="pt", tag="pt")
            nc.tensor.transpose(pt[:], attn[:], ident[:])
            attnT = smx.tile([96, 96], bf16, name="attnT", tag="attnT")
            nc.scalar.copy(attnT[:], pt[:])
            # tmp[d, h2, u, c] = attnT[d, c] * tv[d, h2, u]
            tmp = smx.tile([96, 2, 9, 96], bf16, name="tmp", tag="tmp")
            for h2 in range(2):
                for u in range(9):
                    nc.vector.tensor_scalar(tmp[:, h2, u], attnT[:],
                                            dv_t[:, h2, u:u + 1], None,
                                            OP.mult)
            # C_u[m, c] via matmul: out[m_j, (u, c)] = sum_d Wv[d, m_j] tmp[d, u, c]
            for h2 in range(2):
                for j in range(2):
                    ca = bps.tile([96, 5, 96], f32, name="ca", tag="ca")
                    cb = bps.tile([96, 4, 96], f32, name="cb", tag="cb")
                    nc.tensor.matmul(ca[:], wv_dm[:, h2, j, :],
                                     tmp[:, h2, 0:5, :], start=True, stop=True)
                    nc.tensor.matmul(cb[:], wv_dm[:, h2, j, :],
                                     tmp[:, h2, 5:9, :], start=True, stop=True)
                    nc.scalar.mul(C8[:, h2, j, 0:5, :], ca[:], SCU)
                    nc.scalar.mul(C8[:, h2, j, 5:9, :], cb[:], SCU)
        scp_ctx.close()

        # ================= PHASE C =================
        with tc.tile_pool(name="yps", bufs=2, space="PSUM") as yps, \
             tc.tile_pool(name="sps", bufs=1, space="PSUM") as sps, \
             tc.tile_pool(name="rps", bufs=1, space="PSUM") as rps, \
             tc.tile_pool(name="pps", bufs=1, space="PSUM") as pps, \
             tc.tile_pool(name="ysp", bufs=2) as ysp, \
             tc.tile_pool(name="osp", bufs=2) as osp:
            for c in range(NCHUNK):
                r = RPC * c
                off = 512 * c
                y_ps = yps.tile([96, 2, RPC, 128], f32, name="yps", tag="yps")
                for h2 in range(2):
                    for u in range(9):
                        dy, dx = OFFS[u]
                        nc.tensor.matmul(
                            y_ps[:, h2],
                            C8[:, h2, :, u, :],
                            xs[:, :, 1 + r + dy:1 + r + dy + RPC,
                               1 + dx:1 + dx + W],
                            start=(u == 0), stop=(u == 8), perf_mode=DR)
                ysb = ysp.tile([96, 2, RPC, 128], f8, name="ysb", tag="ysb")
                for h2 in range(2):
                    nc.scalar.mul(ysb[:, h2], y_ps[:, h2], SY)
                yy = ysp.tile([96, 2, RPC, 128], bf16, name="yy", tag="yy")
                nc.vector.tensor_tensor(yy[:], ysb[:], ysb[:], OP.mult)
                pss = sps.tile([1, RPC, 128], f32, name="pss", tag="pss")
                nc.tensor.matmul(pss[:], ones96[:], yy[:, 0], start=True,
                                 stop=False)
                nc.tensor.matmul(pss[:], ones96[:], yy[:, 1], start=False,
                                 stop=True)
                rsb = osp.tile([1, RPC, 128], bf16, name="rsb", tag="rsb")
                nc.scalar.activation(rsb[:], pss[:], AF.Abs_reciprocal_sqrt,
                                     bias=eps[0:1, 0:1], scale=RSQ_SCALE)
                rbsb = rps.tile([96, RPC, 128], f32, name="rb", tag="rb")
                nc.tensor.matmul(rbsb[:], onesF[:], rsb[:], start=True,
                                 stop=True)
                rb_sb = osp.tile([96, RPC, 128], bf16, name="rbs", tag="rbs")
                nc.scalar.copy(rb_sb[:], rbsb[:])
                po = pps.tile([96, 2, RPC, 128], f32, name="po", tag="po")
                for oc in range(2):
                    nc.tensor.matmul(po[:, oc], wo8[:, :, oc, :], ysb[:],
                                     start=True, stop=True, perf_mode=DR)
                osb = osp.tile([96, 2, RPC, 128], bf16, name="osb", tag="osb")
                nc.vector.tensor_tensor(osb[:, 0], po[:, 0], rb_sb[:], OP.mult)
                nc.vector.tensor_tensor(osb[:, 1], po[:, 1], rb_sb[:], OP.mult)
                for oc in range(2):
                    nc.sync.dma_start(out_d[oc, :, off:off + 512], osb[:, oc])
    nc.compile()
    return nc


def _prep_inputs(inputs):
    x = np.asarray(inputs["x"], np.float32)
    norm_w = np.asarray(inputs["norm_w"], np.float32)
    Wq = np.asarray(inputs["Wq"], np.float32)
    Wk = np.asarray(inputs["Wk"], np.float32)
    Wv = np.asarray(inputs["Wv"], np.float32)
    Dq = np.asarray(inputs["Dq"], np.float32)
    Dk = np.asarray(inputs["Dk"], np.float32)
    Dv = np.asarray(inputs["Dv"], np.float32)
    t1 = np.asarray(inputs["t1"], np.float32)
    t2 = np.asarray(inputs["t2"], np.float32)
    hn_w = np.asarray(inputs["hn_w"], np.float32)
    Wo = np.asarray(inputs["Wo"], np.float32)
    lam = float(np.exp(np.sum(inputs["lq1"] * inputs["lk1"], dtype=np.float64))
                - np.exp(np.sum(inputs["lq2"] * inputs["lk2"], dtype=np.float64))
                + LAM_INIT)

    var = x.var(axis=1)
    s = 1.0 / np.sqrt(var + 1e-5)
    xs = (x * s[:, None, :, :]) * SXS          # [B, 192, H, W]
    # [B, 96, 2, H, W] fp8  (partition c96, k-tile j)
    xs8 = np.ascontiguousarray(
        xs.reshape(B, 2, 96, H, W).transpose(0, 2, 1, 3, 4)).astype(F8)

    Wq_f = Wq * norm_w[None, :]
    Wk_f = Wk * norm_w[None, :]
    Wv_f = Wv * norm_w[None, :]

    in_maps = []
    for core in range(8):
        b, h = core // 2, core % 2
        sl = slice(h * 192, (h + 1) * 192)
        m = {}
        m["xs8"] = xs8[b]
        dq = Dq[sl, 0].reshape(192, 9).copy()
        dk = Dk[sl, 0].reshape(192, 9).copy()
        dvv = Dv[sl, 0].reshape(192, 9)
        tq_s = np.sqrt(np.float32(t1[h, 0, 0]))
        tk_s = np.sqrt(np.float32(t2[h, 0, 0]))
        dq[:96] *= tq_s
        dq[96:] *= tk_s
        dk[:96] *= tq_s
        dk[96:] *= tk_s
        # w8[c96, t, j, hf, o96] = W_f[o, c] * taps[o, t] * SW, c = 96j + c96
        for nm_, Wf, taps in (("wq8", Wq_f[sl], dq), ("wk8", Wk_f[sl], dk)):
            wt = (Wf[None, :, :] * taps.T[:, :, None] * SW)   # [9t, 192o, 192c]
            wt = wt.reshape(9, 2, 96, 2, 96)                  # t, hf, o96, j, c96
            m[nm_] = np.ascontiguousarray(
                wt.transpose(4, 0, 3, 1, 2)).astype(F8)       # c96,t,j,hf,o96
        # wv_dm[d, h2, j, m96] = Wv_f[sl][96*h2 + d, 96*j + m96]
        m["wv_dm"] = np.ascontiguousarray(
            Wv_f[sl].reshape(2, 96, 2, 96).transpose(1, 0, 2, 3)).astype(BF16)
        # dv_t[d, h2, u]
        m["dv_t"] = np.ascontiguousarray(
            dvv.reshape(2, 96, 9).transpose(1, 0, 2)).astype(np.float32)
        # wo8[y96, j, oc, o96] = Wo_h[96*oc + o96, 96*j + y96] * SWO
        Wo_h = Wo[:, sl] * (hn_w[h] * (1.0 - LAM_INIT))[None, :]  # [192o,192y]
        m["wo8"] = np.ascontiguousarray(
            (Wo_h * SWO).reshape(2, 96, 2, 96).transpose(3, 2, 0, 1)).astype(F8)
        m["ident"] = np.eye(96, dtype=BF16)
        m["ones96"] = np.ones((96, 1), BF16)
        m["onesF"] = np.full((1, 96), FVAL, BF16)
        m["neglam"] = np.full((96, 1), -lam, np.float32)
        m["eps"] = np.full((1, 1), 1e-6, np.float32)
        in_maps.append(m)
    return in_maps


def kernel(**inputs):
    from concourse import bass_utils

    if "nc" not in _CACHED:
        _CACHED["nc"] = _build_program()
    nc = _CACHED["nc"]

    in_maps = _prep_inputs(inputs)
    results = bass_utils.run_bass_kernel_spmd(
        nc, in_maps, core_ids=list(range(8))).results

    x = np.asarray(inputs["x"], np.float32)
    out = np.empty((B, C, N), np.float32)
    for b in range(B):
        o0 = results[2 * b]["out"].astype(np.float32).reshape(C, N)
        o1 = results[2 * b + 1]["out"].astype(np.float32).reshape(C, N)
        out[b] = o0 + o1
    out = out.reshape(B, C, H, W) + x
    return out.astype(np.float32)
